# revision 3
# baseline (speedup 1.0000x reference)
"""Trainium2 Bass kernel for nn_KineticEquation (gnn_message_passing), v2.

Reference computation:
    contrib_1 = y[:, i1r] * rate1                 # [B, R1]
    contrib_2 = y[:, i2r0] * y[:, i2r1] * rate2   # [B, R2]
    y_out = scatter_add(contrib_1 -> i1p) + scatter_add(contrib_2 -> i2p)

Strategy (8 NeuronCores, full batch per core, reactions sharded by product
tile p//128 so each core owns one 128-row slice of y_out^T):

  * First-order term: dense bf16 matmul y_out1^T = W1^T @ y^T (8 PE matmuls).
  * Second-order reactions in chunks of <=128 sharing (T0, T1) = (r0//128,
    r1//128). Gathers use fp8 DoubleRow matmuls whose two k-tiles hold a
    hi/lo fp8 decomposition of the gathered table, with the one-hot weights
    broadcast across the k-pair via a stride-0 AP (so hi and lo use the same
    one-hot and the gather returns the full ~12-bit value in one 0.5
    cycle/row matmul).

    Two per-chunk routes keep ACT and DVE both busy:
      route A (log): s = L[r0] + L[r1] accumulated into ONE PSUM tile by two
        DR matmuls (L = log y, fp8 hi/lo); z = Exp(s + log rate) on ACT
        (single PSUM->SBUF crossing), out fp8.
      route C (lin): g0 = y[r0], g1 = y[r1] via two DR matmuls; DVE copies
        g0 to SBUF, then one fused DVE op z = (g0 * rate) * g1 -> fp8.

    Chunk pairs share one fp8 DoubleRow scatter matmul into the persistent
    PSUM accumulator (k-tile = which chunk of the pair).
"""

import math

import numpy as np

import concourse.tile as tile
from concourse import bacc, mybir
from concourse.bass import AP
from concourse.bass_utils import run_bass_kernel_spmd

F32 = mybir.dt.float32
BF16 = mybir.dt.bfloat16
FP8 = mybir.dt.float8e4
DR = mybir.MatmulPerfMode.DoubleRow

NCORES = 8
P = 128           # partitions / tile edge
S = 1024          # species
NT = S // P       # species tiles (8)
B = 512           # batch
GRP = 8           # chunks per DMA group
A_NUM, A_DEN = 1, 1   # fraction of chunks on the exp route (ACT)
LAGP = 3          # scatter lag, in chunk pairs


def _route_is_A(c):
    return (c * A_NUM) % A_DEN < A_NUM


def _q8(x, E4):
    return x.astype(E4).astype(np.float32)


def _preprocess(y_in, i1r, i1p, r1, i2r0, i2r1, i2p, r2):
    """Host-side index preprocessing. Returns per-core input dicts + schedule."""
    E4 = mybir.dt.np(FP8)
    NBF = mybir.dt.np(BF16)

    i1r = np.asarray(i1r).astype(np.int64)
    i1p = np.asarray(i1p).astype(np.int64)
    i2r0 = np.asarray(i2r0).astype(np.int64)
    i2r1 = np.asarray(i2r1).astype(np.int64)
    i2p = np.asarray(i2p).astype(np.int64)
    r1 = np.asarray(r1).astype(np.float32)
    r2 = np.asarray(r2).astype(np.float32)

    # Dense first-order matrix W1[s, p] = sum of rates
    W1 = np.zeros((S, S), np.float32)
    np.add.at(W1, (i1r, i1p), r1)

    yT = np.ascontiguousarray(np.asarray(y_in, np.float32).T)  # [S, B]

    # hi/lo fp8 decompositions of y^T and log(y^T), interleaved per tile:
    # X8[t] = [128, 2, 512] with [:, 0, :] = hi, [:, 1, :] = lo
    def hilo(M):
        hi = M.astype(E4)
        lo = (M - hi.astype(np.float32)).astype(E4)
        out = np.empty((NT, P, 2, B), E4)
        for t in range(NT):
            out[t, :, 0, :] = hi[t * P:(t + 1) * P]
            out[t, :, 1, :] = lo[t * P:(t + 1) * P]
        return out.reshape(NT, P, 2 * B)

    Y8 = hilo(yT)
    L8 = hilo(np.log(np.maximum(yT, 1e-30)))
    Yb = np.ascontiguousarray(yT.reshape(NT, P, B)).astype(NBF)

    # Shard second-order reactions by product tile. Log-space gathers are
    # commutative, so bin by the UNORDERED tile pair (36 bins, better fill):
    # the operand in the lower tile goes through G0, the other through G1.
    core_of = i2p >> 7
    T0 = i2r0 >> 7
    T1 = i2r1 >> 7
    swap = T0 > T1
    Ta = np.minimum(T0, T1)
    Tb = np.maximum(T0, T1)
    binid = (Ta << 3) | Tb

    counts = np.zeros((NCORES, NT * NT), np.int64)
    for c in range(NCORES):
        counts[c] = np.bincount(binid[core_of == c], minlength=NT * NT)
    maxc = counts.max(axis=0)
    nch_b = np.ceil(maxc / P).astype(np.int64)          # chunks per bin
    base_b = np.zeros(NT * NT + 1, np.int64)
    base_b[1:] = np.cumsum(nch_b)
    nchunk = int(base_b[-1])
    nchunk += nchunk & 1                                # pad to even (pairs)
    ngroup = math.ceil(nchunk / GRP)
    nchpad = ngroup * GRP

    sched = []  # (T0, T1, is_A) per chunk
    for b in range(NT * NT):
        for _ in range(int(nch_b[b])):
            sched.append((b >> 3, b & 7, _route_is_A(len(sched))))
    while len(sched) < nchunk:
        sched.append((0, 0, _route_is_A(len(sched))))

    lograte_pad = np.float32(-100.0)

    in_maps = []
    for c in range(NCORES):
        sel = core_of == c
        bsel = binid[sel]
        order = np.argsort(bsel, kind="stable")
        bs = bsel[order]
        sw = swap[sel][order]
        r0l_ = (i2r0[sel] & 127)[order]
        r1l_ = (i2r1[sel] & 127)[order]
        r0l = np.where(sw, r1l_, r0l_)
        r1l = np.where(sw, r0l_, r1l_)
        pl = (i2p[sel] & 127)[order]
        rr = r2[sel][order]
        bin_start = np.zeros(NT * NT, np.int64)
        cnt = np.bincount(bs, minlength=NT * NT)
        bin_start[1:] = np.cumsum(cnt)[:-1]
        pos = np.arange(len(bs)) - bin_start[bs]
        chunk = base_b[bs] + (pos >> 7)
        col = pos & 127

        G0 = np.zeros((nchpad, P, P), np.float32)
        G1 = np.zeros((nchpad, P, P), np.float32)
        SC = np.zeros((nchpad, P, P), np.float32)
        diag_r = (bs >> 3) == (bs & 7)          # reactions in diagonal bins
        G0[chunk[~diag_r], r0l[~diag_r], col[~diag_r]] = 1.0
        G1[chunk[~diag_r], r1l[~diag_r], col[~diag_r]] = 1.0
        # diagonal bins: two-hot in G0 (log route: L[r0]+L[r1], one matmul);
        # r0 == r1 accumulates to 2.0 = log(y^2), exactly right.
        np.add.at(G0, (chunk[diag_r], r0l[diag_r], col[diag_r]), 1.0)
        np.add.at(G0, (chunk[diag_r], r1l[diag_r], col[diag_r]), 1.0)
        SC[chunk, col, pl] = 1.0

        # per-chunk per-slot rate values: log(rate) for route A, rate for C
        RT = np.zeros((nchpad, P), np.float32)
        isA = np.array([s[2] for s in sched] + [False] * (nchpad - nchunk))
        RT[isA] = lograte_pad
        ch_isA = isA[chunk]
        RT[chunk[ch_isA], col[ch_isA]] = np.log(np.maximum(rr[ch_isA], 1e-30))
        RT[chunk[~ch_isA], col[~ch_isA]] = rr[~ch_isA]

        def grp(x):
            # [nchpad, P, P] -> [ngroup, P, GRP*P], chunk-major columns
            return np.ascontiguousarray(
                x.reshape(ngroup, GRP, P, P).transpose(0, 2, 1, 3)
                .reshape(ngroup, P, GRP * P)
            )

        in_maps.append(
            dict(
                Y8=Y8,
                L8=L8,
                Yb=Yb,
                W1g=np.ascontiguousarray(
                    W1[:, c * P:(c + 1) * P].reshape(NT, P, P)).astype(NBF),
                G0=grp(G0).astype(E4),
                G1=grp(G1).astype(E4),
                SCT=grp(SC).astype(E4),
                RT=np.ascontiguousarray(
                    RT.reshape(ngroup, GRP, P).transpose(0, 2, 1)).copy(),
            )
        )
    return in_maps, sched, nchunk, ngroup


def _bc2(ap):
    """lhsT one-hot broadcast across the DoubleRow k-pair: [128, 128] ->
    [128, (0-stride k=2), 128]."""
    return AP(ap.tensor, ap.offset, [ap.ap[0], [0, 2], ap.ap[1]])


def _k2(ap, w):
    """view a [128, 2*w] AP as [128, (k=2, stride w), w] for DoubleRow."""
    return AP(ap.tensor, ap.offset, [ap.ap[0], [w, 2], [1, w]])


def _build(nchunk, ngroup, sched, reps=1, warmup=12, npre=2, lagp=LAGP,
           skip=(), force_route=None):
    nc = bacc.Bacc("TRN2", target_bir_lowering=False, debug=False,
                   num_devices=NCORES)

    y8_d = nc.dram_tensor("Y8", [NT, P, 2 * B], FP8, kind="ExternalInput").ap()
    l8_d = nc.dram_tensor("L8", [NT, P, 2 * B], FP8, kind="ExternalInput").ap()
    yb_d = nc.dram_tensor("Yb", [NT, P, B], BF16, kind="ExternalInput").ap()
    w1_d = nc.dram_tensor("W1g", [NT, P, P], BF16, kind="ExternalInput").ap()
    g0_d = nc.dram_tensor("G0", [ngroup, P, GRP * P], FP8, kind="ExternalInput").ap()
    g1_d = nc.dram_tensor("G1", [ngroup, P, GRP * P], FP8, kind="ExternalInput").ap()
    sc_d = nc.dram_tensor("SCT", [ngroup, P, GRP * P], FP8, kind="ExternalInput").ap()
    rt_d = nc.dram_tensor("RT", [ngroup, P, GRP], F32, kind="ExternalInput").ap()
    out_d = nc.dram_tensor("out", [P, B], F32, kind="ExternalOutput").ap()

    EXP = mybir.ActivationFunctionType.Exp
    MUL = mybir.AluOpType.mult
    has_C = (force_route == "C") or (force_route is None and
                                     any(not s[2] for s in sched[:nchunk]))
    sa_bufs = 5 if has_C else 7

    with tile.TileContext(nc) as tc:
        from contextlib import ExitStack
        with ExitStack() as stack:
            res = stack.enter_context(tc.tile_pool(name="res", bufs=1))
            ohp = stack.enter_context(tc.tile_pool(name="oh", bufs=4))
            wp = stack.enter_context(tc.tile_pool(name="work", bufs=3))
            zpp = stack.enter_context(tc.tile_pool(name="zp", bufs=3))
            accp = stack.enter_context(tc.tile_pool(name="acc", bufs=1, space="PSUM"))
            sAp = stack.enter_context(tc.tile_pool(name="sA", bufs=sa_bufs, space="PSUM"))
            gp0p = gp1p = None
            if has_C:
                gp0p = stack.enter_context(tc.tile_pool(name="gp0", bufs=1, space="PSUM"))
                gp1p = stack.enter_context(tc.tile_pool(name="gp1", bufs=1, space="PSUM"))
            # PE warmup: dependency-free matmuls during the initial DMA
            # window so the clock ramps before real work.
            if warmup:
                wt = res.tile([P, P], BF16, tag="warm")
                nc.vector.memset(wt[:], 0.0)
                for _ in range(warmup):
                    wps = sAp.tile([P, B], F32, space="PSUM", tag="sA")
                    nc.tensor.matmul(wps[:, :8], lhsT=wt[:], rhs=wt[:, :8],
                                     start=True, stop=True)

            # Pre-issue the first one-hot groups ahead of the residents.
            pre = []
            for gi in range(min(npre, ngroup)):
                pg0 = ohp.tile([P, GRP * P], FP8, tag="g0g")
                pg1 = ohp.tile([P, GRP * P], FP8, tag="g1g")
                psc = ohp.tile([P, GRP * P], FP8, tag="scg")
                prt = ohp.tile([P, GRP], F32, tag="rtg")
                nc.sync.dma_start(pg0[:], g0_d[gi])
                nc.sync.dma_start(pg1[:], g1_d[gi])
                nc.sync.dma_start(psc[:], sc_d[gi])
                nc.sync.dma_start(prt[:], rt_d[gi])
                pre.append((pg0, pg1, psc, prt))

            # Resident tables
            lts, yts, ybs = [], [], []
            for t in range(NT):
                lt = res.tile([P, 2 * B], FP8, tag=f"l{t}")
                nc.sync.dma_start(lt[:], l8_d[t])
                lts.append(lt)
            if has_C:
                for t in range(NT):
                    yt = res.tile([P, 2 * B], FP8, tag=f"y8{t}")
                    nc.sync.dma_start(yt[:], y8_d[t])
                    yts.append(yt)
            for t in range(NT):
                yb = res.tile([P, B], BF16, tag=f"yb{t}")
                nc.sync.dma_start(yb[:], yb_d[t])
                ybs.append(yb)
            w1t = res.tile([P, NT * P], BF16, tag="w1")
            for t in range(NT):
                nc.sync.dma_start(w1t[:, t * P:(t + 1) * P], w1_d[t])

            def one_pass(first_pass):
                from collections import deque
                acc = accp.tile([P, B], F32, space="PSUM", tag="acc")
                first_acc = [True]

                def acc_mm(lhsT, rhs, stop=False, perf_mode=None):
                    nc.tensor.matmul(acc[:], lhsT=lhsT, rhs=rhs,
                                     start=first_acc[0], stop=stop,
                                     perf_mode=perf_mode, skip_group_check=True)
                    first_acc[0] = False

                pending = deque()
                zp = None
                for c in range(nchunk):
                    t0, t1, isA = sched[c]
                    if force_route is not None:
                        isA = force_route == "A"
                    gi, k = divmod(c, GRP)
                    if k == 0:
                        if pre and first_pass and gi < len(pre):
                            g0g, g1g, scg, rtg = pre[gi]
                        else:
                            g0g = ohp.tile([P, GRP * P], FP8, tag="g0g")
                            g1g = ohp.tile([P, GRP * P], FP8, tag="g1g")
                            scg = ohp.tile([P, GRP * P], FP8, tag="scg")
                            rtg = ohp.tile([P, GRP], F32, tag="rtg")
                            nc.sync.dma_start(g0g[:], g0_d[gi])
                            nc.sync.dma_start(g1g[:], g1_d[gi])
                            nc.sync.dma_start(scg[:], sc_d[gi])
                            nc.sync.dma_start(rtg[:], rt_d[gi])
                    cs = slice(k * P, (k + 1) * P)
                    kt = c & 1
                    if kt == 0:
                        zp = zpp.tile([P, 2 * B], FP8, tag="zp")

                    lg0 = _bc2(g0g[:, cs])
                    lg1 = _bc2(g1g[:, cs])
                    rate = rtg[:, k:k + 1]
                    zslice = zp[:, kt * B:(kt + 1) * B]
                    if "ew" in skip:
                        if isA:
                            sp = sAp.tile([P, B], F32, space="PSUM", tag="sA")
                            nc.tensor.matmul(sp[:], lhsT=lg0, rhs=_k2(lts[t0][:], B),
                                             start=True, stop=(t0 == t1), perf_mode=DR)
                            if t0 != t1:
                                nc.tensor.matmul(sp[:], lhsT=lg1, rhs=_k2(lts[t1][:], B),
                                                 start=False, stop=True, perf_mode=DR)
                        else:
                            g0p = gp0p.tile([P, B], F32, space="PSUM", tag="g0p")
                            g1p = gp1p.tile([P, B], F32, space="PSUM", tag="g1p")
                            nc.tensor.matmul(g0p[:], lhsT=lg0, rhs=_k2(yts[t0][:], B),
                                             start=True, stop=True, perf_mode=DR)
                            nc.tensor.matmul(g1p[:], lhsT=lg1, rhs=_k2(yts[t1][:], B),
                                             start=True, stop=True, perf_mode=DR)
                        continue
                    if isA:
                        sp = sAp.tile([P, B], F32, space="PSUM", tag="sA")
                        nc.tensor.matmul(sp[:], lhsT=lg0, rhs=_k2(lts[t0][:], B),
                                         start=True, stop=(t0 == t1), perf_mode=DR)
                        if t0 != t1:
                            nc.tensor.matmul(sp[:], lhsT=lg1, rhs=_k2(lts[t1][:], B),
                                             start=False, stop=True, perf_mode=DR)
                        nc.scalar.activation(zslice, sp[:], EXP,
                                             bias=rate, scale=1.0)
                    else:
                        g0p = gp0p.tile([P, B], F32, space="PSUM", tag="g0p")
                        g1p = gp1p.tile([P, B], F32, space="PSUM", tag="g1p")
                        nc.tensor.matmul(g0p[:], lhsT=lg0, rhs=_k2(yts[t0][:], B),
                                         start=True, stop=True, perf_mode=DR)
                        nc.tensor.matmul(g1p[:], lhsT=lg1, rhs=_k2(yts[t1][:], B),
                                         start=True, stop=True, perf_mode=DR)
                        g0s = wp.tile([P, B], F32, tag="g0s")
                        nc.vector.tensor_copy(g0s[:], g0p[:])
                        nc.vector.scalar_tensor_tensor(
                            out=zslice, in0=g0s[:], scalar=rate,
                            in1=g1p[:], op0=MUL, op1=MUL)

                    if kt == 1:
                        if "scatter" in skip:
                            continue
                        jj = (c % GRP) // 2
                        sc2 = _k2(scg[:, 2 * jj * P:(2 * jj + 2) * P], P)
                        pending.append((sc2, zp))
                        if len(pending) > lagp:
                            lh, zz = pending.popleft()
                            acc_mm(lh, _k2(zz[:], B), perf_mode=DR)

                while pending:
                    lh, zz = pending.popleft()
                    acc_mm(lh, _k2(zz[:], B), perf_mode=DR)

                # First-order term last: acc += sum_t W1g[t]^T @ y^T[t]
                for t in range(NT):
                    acc_mm(w1t[:, t * P:(t + 1) * P], ybs[t][:],
                           stop=(t == NT - 1))

                outs = wp.tile([P, B], F32, tag="outs")
                nc.vector.tensor_copy(outs[:], acc[:])
                nc.sync.dma_start(out_d[:], outs[:])

            for rep in range(reps):
                one_pass(rep == 0)

    nc.compile()
    return nc


def _run(inputs, trace=False):
    in_maps, sched, nchunk, ngroup = _preprocess(
        inputs["y_in"], inputs["inds_1r"], inputs["inds_1p"], inputs["rate_1"],
        inputs["inds_2r0"], inputs["inds_2r1"], inputs["inds_2p"], inputs["rate_2"],
    )
    nc = _build(nchunk, ngroup, sched)
    res = None
    y_out = None
    last_exc = None
    for attempt in range(3):
        try:
            res = run_bass_kernel_spmd(nc, in_maps, list(range(NCORES)), trace=trace)
        except Exception as e:  # transient device wedges (NRT_EXEC_UNIT_...)
            last_exc = e
            import time as _time
            _time.sleep(2.0)
            continue
        y_out = np.empty((B, S), np.float32)
        for c in range(NCORES):
            y_out[:, c * P:(c + 1) * P] = res.results[c]["out"].T
        # guard against silent corruption from a wedged device
        if np.isfinite(y_out).all() and not (y_out == 0).all():
            break
        y_out = None
    if y_out is None:
        if last_exc is not None:
            raise last_exc
        raise RuntimeError("kernel produced non-finite/empty output on all attempts")
    return y_out, res


def kernel(**inputs) -> np.ndarray:
    return _run(inputs, trace=False)[0]
